# revision 50
# baseline (speedup 1.0000x reference)
"""Trainium2 Bass kernel for nn_Attention_39573828665647.

GQA causal attention block (B=4, S=1024, DIM=2048, 32 q heads / 8 kv heads,
hd=64) with RoPE, sharded over 8 NeuronCores as (batch x head-half):
core = 2*b + hh handles batch b and kv groups [4hh, 4hh+4) (16 q heads).
Each core computes a partial output projection over its 1024 o-dims; the
host sums the two partials per batch.

v3 design (over the v2 baseline, HW 303us -> 239us; TimelineSim 292 -> 222):
  * Denominator via wide ones-block: vaug is [128, 4, 128] (per group g,
    cols 0:64 = v_g, 64:128 = ones), so the AV matmul's [128,128]
    stationary emits o on psum rows 0:64 and the softmax denominator
    replicated on rows 64:128 for free (matmul time = moving cols).
    normalize is then reciprocal + mul on DVE with NO DRAM round-trip /
    broadcast DMAs (v2 spent ~1us of issuing-engine time per trigger and
    blocked the SP/Pool queue heads at every head-pair boundary).
  * Output stores coalesced into [128, 1024] halves staged per seq-block,
    alternating the SP and ACT HWDGE rings: per-ring HBM-write completion
    receipts (~2us each, FIFO per ring) made 32 small stores cost ~40us of
    serialized tail on HW (invisible in the cost model).
  * ACT load smoothing: slots j<4 compute only the t0 score piece of key
    j; slot j>=4 computes key (j-4)'s t1 piece plus key j's piece, and the
    t1 AV accumulates per-slot. exp work per slot is then ~even and the AV
    never waits on a block-start exp burst.
  * All DMA triggers ride SP and ACT (HWDGE) plus Pool for the wkv evens
    in phase 0; DVE and (during compute) Pool never issue DMAs, keeping
    rope, affine select, and normalize off the trigger path. Coarse load
    tiles: x and wq full-width per d-tile, wo full-width per d'-tile.
  * Score matmuls contract over hd=64 only; even/odd heads of a pair sit
    at partitions 0:64 / 64:128, so their MMs land on disjoint PE row
    groups; emission interleaves the heads (ae, ao, be, bo) so adjacent
    MMs run concurrently in the array.
  * q-proj emission is metered (FILLER) to front-load exp-independent PE
    work into the ACT-heavier early slots; rope-k runs on a DMA-free
    DVE/Pool so the q-proj psum-bank WAR clears just after v-proj ends.
"""

import numpy as np
import ml_dtypes

B, S, DIM = 4, 1024, 2048
NH, NKV, HD = 32, 8, 64
P = 128
ND = DIM // P  # 16 d-tiles

_SWAP_ADJ = [i ^ 1 for i in range(32)]  # pairwise partition swap within quadrants

_CACHE = {}


def host_prep(x, freqs_cos, freqs_sin, wqkv, wo):
    """Build the 8 per-core input dicts (bf16 weights/activations)."""
    bf16 = ml_dtypes.bfloat16
    x = np.asarray(x, np.float32)
    wqkv = np.asarray(wqkv, np.float32)
    wo = np.asarray(wo, np.float32)
    cos = np.asarray(freqs_cos, np.float32)
    sin = np.asarray(freqs_sin, np.float32)

    cosT, sinT = cos.T, sin.T                      # [32, S]
    C64 = np.repeat(cosT, 2, axis=0)               # [64, S]
    Ss64 = np.repeat(sinT, 2, axis=0).copy()
    Ss64[0::2] *= -1.0                             # even rows -sin, odd +sin
    C64 = np.ascontiguousarray(C64, dtype=np.float32)
    Ss64 = np.ascontiguousarray(Ss64, dtype=np.float32)
    scale = np.float32(1.0 / np.sqrt(HD))
    Cq, Sq = C64 * scale, Ss64 * scale      # [64, S]; kernel duplicates rows
    Ck, Sk = C64, Ss64

    woT_full = np.ascontiguousarray(wo.T.astype(bf16))          # [d', o]
    xT_full = np.ascontiguousarray(
        x.transpose(0, 2, 1).astype(bf16))  # [B, DIM, S]
    wqkvT_full = np.ascontiguousarray(wqkv.T.astype(bf16))      # [DIM, 3072]
    in_maps = []
    for core in range(8):
        b, hh = core // 2, core % 2
        groups = range(4 * hh, 4 * hh + 4)
        qheads = range(16 * hh, 16 * hh + 16)
        # column order: k groups (256) | v groups (256) | q heads (1024)
        wqkvT = np.empty((DIM, 1536), bf16)
        col = 0
        blocks = ([(g * 6 + 4) * 64 for g in groups]
                  + [(g * 6 + 5) * 64 for g in groups]
                  + [(h // 4 * 6 + h % 4) * 64 for h in qheads])
        for c0 in blocks:
            wqkvT[:, col:col + 64] = wqkvT_full[:, c0:c0 + 64]
            col += 64
        in_maps.append({
            "xT": xT_full[b],                                      # [2048, 1024]
            "wqkvT": wqkvT,                                        # [2048, 1536]
            "woT": np.ascontiguousarray(woT_full[1024 * hh:1024 * hh + 1024]),
            "Cq": Cq.astype(bf16), "Sq": Sq.astype(bf16),
            "Ck": Ck.astype(bf16), "Sk": Sk.astype(bf16),
        })
    return in_maps


def build_nc(reps=1):
    from contextlib import ExitStack
    import concourse.bacc as bacc
    import concourse.bass as bass
    import concourse.tile as tile
    import concourse.mybir as mybir

    f32 = mybir.dt.float32
    bf16 = mybir.dt.bfloat16
    EXP = mybir.ActivationFunctionType.Exp

    nc = bacc.Bacc("TRN2", target_bir_lowering=False, debug=False)
    xT_d = nc.dram_tensor("xT", [DIM, S], bf16, kind="ExternalInput")
    wqkvT_d = nc.dram_tensor("wqkvT", [DIM, 1536], bf16, kind="ExternalInput")
    woT_d = nc.dram_tensor("woT", [1024, DIM], bf16, kind="ExternalInput")
    Cq_d = nc.dram_tensor("Cq", [64, S], bf16, kind="ExternalInput")
    Sq_d = nc.dram_tensor("Sq", [64, S], bf16, kind="ExternalInput")
    Ck_d = nc.dram_tensor("Ck", [64, S], bf16, kind="ExternalInput")
    Sk_d = nc.dram_tensor("Sk", [64, S], bf16, kind="ExternalInput")
    out_d = nc.dram_tensor("out", [S, DIM], bf16, kind="ExternalOutput")

    import os
    cut = os.environ.get("KERNEL_CUT", "full")  # dev-only phase timing

    def emit(tc, pfx):
        with ExitStack() as stack:
            resid = stack.enter_context(tc.tile_pool(name=pfx + "resid", bufs=1))

            def rtile(shape, dt_, nm):
                return resid.tile(shape, dt_, tag=pfx + nm, name=pfx + nm)

            # q/k/o tiles split into 512-col halves: tile-granular dependency
            # tracking then lets consumers of one half start without waiting
            # for the writer of the other half.
            q_sb = {(i, t): rtile([P, 512], bf16, f"q{i}_{t}")
                    for i in range(8) for t in (0, 1)}
            k_sb = {(g, t): rtile([P, 512], bf16, f"k{g}_{t}")
                    for g in range(4) for t in (0, 1)}
            # vaug: per group g, cols [g, 0:64] = v_g and [g, 64:128] = ones,
            # so the AV matmul's [128,128] stationary emits o on rows 0:64
            # and the softmax denominator replicated on rows 64:128.
            vaug = [rtile([P, 4, 128], bf16, f"va{i}") for i in range(8)]
            o_sb = {(i, t): rtile([P, 512], bf16, f"o{i}_{t}")
                    for i in range(8) for t in (0, 1)}
            wq_sb = [rtile([P, 1024], bf16, f"wq{d}") for d in range(ND)]

            def vaug_st(st, g):
                """[128, 128] stationary AP: v_g cols then ones cols."""
                return vaug[st][:, g, :]

            xres_pool = stack.enter_context(
                tc.tile_pool(name=pfx + "xres", bufs=1))
            rc_pool = stack.enter_context(
                tc.tile_pool(name=pfx + "ropeconst", bufs=1))
            rt_pool = stack.enter_context(
                tc.tile_pool(name=pfx + "ropetmp", bufs=2))

            # ---------------- DMA phase 0 ----------------
            # SP queue: Cq, Sq, x (full-width per d), wo. ACT queue: Ck, Sk,
            # wkv, wq. Pool/DVE never trigger DMAs (SWDGE costs ~1us of
            # issuing-engine time; SP/ACT use HWDGE and are idle here).
            # Table DMAs first (tiny), then x/wkv interleaved so the k-proj
            # d-stream arrives evenly; wkv is split across SP/ACT because a
            # single queue's ~0.6-1us trigger rate would pace the k chains.
            # The [64]->[64:128] dup copies run on DVE/Pool (idle here) so
            # they never stall the trigger queues.
            c_sb = {nm: rc_pool.tile([P, S], bf16, tag=nm, name=pfx + nm)
                    for nm in ("Ck", "Sk", "Cq", "Sq")}
            xres, wkv = [], []
            p1stack = ExitStack()  # phase-1-scoped pools (freed before phase 2)
            w_pool = p1stack.enter_context(tc.tile_pool(name=pfx + "wkv", bufs=1))
            for d in range(ND):
                xt = xres_pool.tile([P, S], bf16, tag=f"x{d}", name=pfx + f"x{d}")
                wt = w_pool.tile([P, 512], bf16, tag=f"wkv{d}",
                                 name=pfx + f"wkv{d}")
                nc.sync.dma_start(out=xt[:], in_=xT_d[d * P:(d + 1) * P, :])
                # wkv split Pool (evens, SWDGE) / ACT (odds): a single
                # queue's trigger rate would pace the k-proj d-stream; Pool
                # and ACT are otherwise idle during phase 0.
                (nc.gpsimd if d % 2 == 0 else nc.scalar).dma_start(
                    out=wt[:], in_=wqkvT_d[d * P:(d + 1) * P, 0:512])
                xres.append(xt)
                wkv.append(wt)
                if d == 7:
                    # rope-k tables ride ACT mid-stream (needed ~15us in,
                    # and ahead of them the wkv odds must not be delayed);
                    # dup copies on DVE
                    for nm, dr in (("Ck", Ck_d), ("Sk", Sk_d)):
                        ct = c_sb[nm]
                        nc.scalar.dma_start(out=ct[0:64, :], in_=dr[:])
                        nc.vector.tensor_copy(ct[64:128, :], ct[0:64, :])
            # rope-q tables on SP after x (needed ~40us in); dup on ACT,
            # which idles between the phase-0 triggers and the first exp —
            # Pool must stay clear for the rope-k muls that gate the psP
            # bank WAR.
            for nm, dr in (("Cq", Cq_d), ("Sq", Sq_d)):
                ct = c_sb[nm]
                nc.sync.dma_start(out=ct[0:64, :], in_=dr[:])
                nc.scalar.copy(ct[64:128, :], ct[0:64, :])
            for d in range(ND):
                nc.sync.dma_start(
                    out=wq_sb[d][:],
                    in_=wqkvT_d[d * P:(d + 1) * P, 512:1536])

            # ---------------- Phase 1: k proj + rope, v proj ----------------
            if True:
                psK = p1stack.enter_context(
                    tc.tile_pool(name=pfx + "psK", bufs=4, space="PSUM"))
                psV = p1stack.enter_context(
                    tc.tile_pool(name=pfx + "psV", bufs=3, space="PSUM"))

                # PE warmup: spin matmuls on const data while first DMAs land
                wmt = rt_pool.tile([P, P], bf16, tag="sh", name=pfx + "wm")
                nc.vector.tensor_copy(wmt[:], nc.const_aps.tensor(0.0, (P, P), f32))
                wps = psV.tile([P, 256], f32, tag="vp", name=pfx + "wps")
                for _ in range(30):
                    nc.tensor.matmul(wps[:, 0:128], wmt[:], wmt[:],
                                     start=True, stop=True)

                # k projection: 4 chains (at,t), d-outer so it starts as the
                # x tiles land; last 4 d-steps chain-major so the rope
                # psum-reads pipeline instead of bunching.
                kps = {}
                for at in (0, 1):
                    for t in (0, 1):
                        kps[(at, t)] = psK.tile([P, 512], f32, tag="kp",
                                                name=pfx + f"kp{at}_{t}")
                # chain (0,0) goes chain-major from d=10 so its rope (which
                # gates the psP bank WAR for block-0 q-proj) finishes before
                # v-proj ends; the others from d=12.
                for d in range(10):
                    for at in (0, 1):
                        for t in (0, 1):
                            nc.tensor.matmul(
                                kps[(at, t)][:],
                                wkv[d][:, at * P:(at + 1) * P],
                                xres[d][:, t * 512:(t + 1) * 512],
                                start=(d == 0), stop=False)
                for d in range(10, ND):
                    nc.tensor.matmul(
                        kps[(0, 0)][:], wkv[d][:, 0:P],
                        xres[d][:, 0:512], start=False, stop=(d == ND - 1))
                for d in range(10, 12):
                    for at in (0, 1):
                        for t in (0, 1):
                            if (at, t) == (0, 0):
                                continue
                            nc.tensor.matmul(
                                kps[(at, t)][:],
                                wkv[d][:, at * P:(at + 1) * P],
                                xres[d][:, t * 512:(t + 1) * 512],
                                start=False, stop=False)
                for at in (0, 1):
                    for t in (0, 1):
                        if (at, t) == (0, 0):
                            continue
                        for d in range(12, ND):
                            nc.tensor.matmul(
                                kps[(at, t)][:],
                                wkv[d][:, at * P:(at + 1) * P],
                                xres[d][:, t * 512:(t + 1) * 512],
                                start=False, stop=(d == ND - 1))

                rope_ctr = [0]

                def rope(ptile, is_q, sl, dst):
                    """dst[:, :] = rope(ptile); sl selects the table columns."""
                    C_ = c_sb["Cq" if is_q else "Ck"]
                    S_ = c_sb["Sq" if is_q else "Sk"]
                    rope_ctr[0] += 1
                    i = rope_ctr[0]
                    sh = rt_pool.tile([P, 512], f32, tag="sh",
                                      name=pfx + f"sh{i}")
                    m1 = rt_pool.tile([P, 512], f32, tag="m1",
                                      name=pfx + f"m1_{i}")
                    m2 = rt_pool.tile([P, 512], f32, tag="m2",
                                      name=pfx + f"m2_{i}")
                    nc.vector.stream_shuffle(sh[:], ptile[:], _SWAP_ADJ)
                    nc.vector.tensor_mul(m1[:], ptile[:], C_[:, sl])
                    nc.gpsimd.tensor_mul(m2[:], sh[:], S_[:, sl])
                    nc.gpsimd.tensor_add(dst[:], m1[:], m2[:])

                # rope k -> k_sb (dup halves so both par-halves see the group)
                ro_pool = p1stack.enter_context(
                    tc.tile_pool(name=pfx + "ro", bufs=2))
                for at in (0, 1):
                    for t in (0, 1):
                        sl = slice(t * 512, t * 512 + 512)
                        ro = ro_pool.tile([P, 512], bf16, tag="ro",
                                          name=pfx + f"ro{at}_{t}")
                        C_ = c_sb["Ck"]
                        S_ = c_sb["Sk"]
                        sh = rt_pool.tile([P, 512], f32, tag="sh",
                                          name=pfx + f"ksh{at}_{t}")
                        m1 = rt_pool.tile([P, 512], f32, tag="m1",
                                          name=pfx + f"km1_{at}_{t}")
                        m2 = rt_pool.tile([P, 512], f32, tag="m2",
                                          name=pfx + f"km2_{at}_{t}")
                        nc.vector.stream_shuffle(sh[:], kps[(at, t)][:], _SWAP_ADJ)
                        nc.vector.tensor_mul(m1[:], kps[(at, t)][:], C_[:, sl])
                        nc.gpsimd.tensor_mul(m2[:], sh[:], S_[:, sl])
                        nc.gpsimd.tensor_add(ro[:], m1[:], m2[:])
                        for half in (0, 1):
                            g = 2 * at + half
                            src = ro[half * 64:half * 64 + 64, :]
                            nc.scalar.copy(k_sb[(g, t)][0:64, :], src)
                            nc.vector.tensor_copy(k_sb[(g, t)][64:128, :], src)

                # v projection (natural layout) + wide ones augmentation
                for st in range(8):
                    pt = psV.tile([P, 256], f32, tag="vp", name=pfx + f"vp{st}")
                    for d in range(ND):
                        nc.tensor.matmul(
                            pt[:], xres[d][:, st * P:(st + 1) * P],
                            wkv[d][:, 256:512], start=(d == 0), stop=(d == ND - 1))
                    if st % 2:
                        nc.vector.tensor_copy(
                            vaug[st][:, :, 64:128],
                            nc.const_aps.tensor(1.0, (P, 4, 64), f32))
                    else:
                        nc.scalar.copy(vaug[st][:, :, 64:128],
                                       nc.const_aps.tensor(1.0, (P, 4, 64), f32))
                    for g in range(4):
                        if g % 2:
                            nc.scalar.copy(
                                vaug[st][:, g, 0:64], pt[:, g * 64:(g + 1) * 64])
                        else:
                            nc.vector.tensor_copy(
                                vaug[st][:, g, 0:64], pt[:, g * 64:(g + 1) * 64])

            p1stack.close()

            if cut == "phase1":
                anchor = resid.tile([P, 512], bf16, tag=pfx + "anchor",
                                    name=pfx + "anchor")
                nc.vector.tensor_copy(anchor[:], k_sb[(0, 0)][:])
                nc.sync.dma_start(out=out_d[0:P, 0:512], in_=anchor[:])
                return

            # ---------------- Phase 2: q proj blocks + attention ----------------
            # wo loads sit in phase 2 (one full-width d'-tile per block, on
            # the SP queue) so the pool reuses the freed wkv space.
            wo_pool = stack.enter_context(tc.tile_pool(name=pfx + "wo", bufs=1))
            wo_sb = {}

            def load_wo(dt_):
                w = wo_pool.tile([P, DIM], bf16, tag=f"wo{dt_}",
                                 name=pfx + f"wo{dt_}")
                nc.sync.dma_start(out=w[:], in_=woT_d[dt_ * P:(dt_ + 1) * P, :])
                wo_sb[dt_] = w

            e_pool = stack.enter_context(tc.tile_pool(name=pfx + "expT", bufs=12))
            n_pool = stack.enter_context(tc.tile_pool(name=pfx + "normtmp", bufs=3))
            ob_pool = stack.enter_context(tc.tile_pool(name=pfx + "outsb", bufs=3))
            psP = stack.enter_context(
                tc.tile_pool(name=pfx + "psumP", bufs=2, space="PSUM"))
            psOp = stack.enter_context(
                tc.tile_pool(name=pfx + "psumO", bufs=4, space="PSUM"))
            psS = stack.enter_context(
                tc.tile_pool(name=pfx + "psumS", bufs=2, space="PSUM"))

            def normalize(h, t, opsum):
                # opsum rows 64:128 = denominator replicated; recip on DVE,
                # scale on Pool (both [64, 512], no DMA / broadcast needed).
                par = h % 2
                rb = n_pool.tile([64, 512], f32, tag="rb",
                                 name=pfx + f"rb{h}_{t}")
                nc.vector.reciprocal(rb[:], opsum[64:128, :])
                dst = o_sb[(h // 2, t)][par * 64:par * 64 + 64, :]
                nc.vector.tensor_mul(dst, opsum[0:64, :], rb[:])

            # --- attention pair machinery (generator-style interleave) ---
            def head_pair_steps(p):
                """Yield per-j emission closures for head pair (2p, 2p+1).

                Each yielded item is (pre_fn, post_fn): pre emits scores+exp
                +affine for slot j, post emits the AV matmuls. The proj
                filler matmuls go between pre and post.
                """
                heads = (2 * p, 2 * p + 1)
                g = p // 2
                opsums = {}
                for h in heads:
                    opsums[(h, 0)] = psOp.tile(
                        [P, 512], f32, tag="op", name=pfx + f"op{h}_0")
                ets = {}

                def pre(j):
                    # Score MMs contract over hd=64 only: even head at
                    # partitions 0:64, odd at 64:128 -> disjoint PE row
                    # groups. Interleave the heads (ae, ao, be, bo) so
                    # consecutive MMs run concurrently in the array.
                    # ACT smoothing: slots j<4 compute only the t0 piece of
                    # key j; slot j>=4 computes key (j-4)'s t1 piece plus
                    # key j's piece, so exp load is ~even across the block.
                    lo = j * P
                    mms = []
                    for h in heads:
                        par = h % 2
                        sl64 = slice(par * 64, par * 64 + 64)
                        qa = q_sb[(h // 2, 0)][sl64, :]
                        qb = q_sb[(h // 2, 1)][sl64, :]
                        if j < 4:
                            et = e_pool.tile([P, S], bf16, tag="et",
                                             name=pfx + f"et{h}_{j}")
                            ets[(h, j)] = et
                            mms.append((h, "a", (g, 0), sl64, lo,
                                        qa[:, lo:512], 512 - lo,
                                        et[:, lo:512]))
                        else:
                            et = e_pool.tile([P, S], bf16, tag="et",
                                             name=pfx + f"et{h}_{j}")
                            ets[(h, j)] = et
                            etA = ets[(h, j - 4)]
                            mms.append((h, "ta", (g, 0), sl64, (j - 4) * P,
                                        qb[:], 512, etA[:, 512:S]))
                            mms.append((h, "b", (g, 1), sl64, lo - 512,
                                        qb[:, lo - 512:512], S - lo,
                                        et[:, lo:S]))
                    # order: (head0, piece0), (head1, piece0), (head0,
                    # piece1), (head1, piece1) — adjacent MMs on disjoint
                    # row groups. (A 4-way 64x64 quarter-packing of the
                    # hd=64 scores was tried and measured WORSE on HW,
                    # +13us: the extra LDW/dispatch cost exceeds any array
                    # concurrency this toolchain achieves.)
                    npiece = len(mms) // 2
                    order = [mms[hh * npiece + pc] for pc in range(npiece)
                             for hh in range(2)]
                    exps = []
                    for h, tag_, kk, sl64, col0, qv, w, et_sl in order:
                        ps = psS.tile([P, w], f32, tag="sp",
                                      name=pfx + f"sp{tag_}{h}_{j}")
                        nc.tensor.matmul(
                            ps[:], k_sb[kk][sl64, col0:col0 + P],
                            qv, start=True, stop=True)
                        exps.append((h, ps, et_sl))
                    for h, ps, et_sl in exps:
                        nc.scalar.activation(et_sl, ps[:], EXP)
                    # diagonal chunk: zero sq < sk (the t1 piece of keys
                    # 0..3 is fully below the diagonal — no mask needed)
                    for h in heads:
                        et = ets[(h, j)]
                        nc.gpsimd.affine_select(
                            out=et[:, lo:lo + P], in_=et[:, lo:lo + P],
                            pattern=[[1, P]], channel_multiplier=-1,
                            base=0, compare_op=mybir.AluOpType.is_ge, fill=0.0)

                def post(j):
                    # t0 opsum accumulates keys 0..3 over slots 0..3; the t1
                    # opsum accumulates keys (j-4) and j at each slot j>=4
                    # (key j-4's t1 exp just ran in pre(j)).
                    lo = j * P
                    for h in heads:
                        if j < 4:
                            et = ets[(h, j)]
                            nc.tensor.matmul(
                                opsums[(h, 0)][:, lo:512], vaug_st(j, g),
                                et[:, lo:512], start=(j == 0), stop=(j == 3))
                            if j == 3:
                                normalize(h, 0, opsums[(h, 0)])
                        else:
                            if j == 4:
                                opsums[(h, 1)] = psOp.tile(
                                    [P, 512], f32, tag="op",
                                    name=pfx + f"op{h}_1")
                            kA = j - 4
                            etA = ets.pop((h, kA))
                            nc.tensor.matmul(
                                opsums[(h, 1)][:, 0:512],
                                vaug_st(kA, g), etA[:, 512:S],
                                start=(j == 4), stop=False)
                            lo1 = lo - 512
                            et = ets.pop((h, j))
                            nc.tensor.matmul(
                                opsums[(h, 1)][:, lo1:512], vaug_st(j, g),
                                et[:, 512 + lo1:S], start=False, stop=(j == 7))

                def finish():
                    for h in heads:
                        normalize(h, 1, opsums[(h, 1)])

                return pre, post, finish

            def proj_chunks(qt):
                """Metered q-proj emission: emit_n(n) doles out n of the 32
                chain matmuls (t0 d0..15 then t1 d0..15), roping each chain
                as it completes. Front-loading the per-slot counts gives PE
                exp-independent filler during the ACT-paced j<4 slots and
                finishes the t1 rope a slot earlier for the next pair's
                pre(0)."""
                qg, a2 = qt // 4, qt % 4
                state = {"i": 0, "pts": {}}

                def emit_n(n):
                    for _ in range(n):
                        i = state["i"]
                        if i >= 32:
                            return
                        t, d = i // 16, i % 16
                        if d == 0:
                            state["pts"][t] = psP.tile(
                                [P, 512], f32, tag="pp",
                                name=pfx + f"pp{qt}_{t}")
                        nc.tensor.matmul(
                            state["pts"][t][:],
                            wq_sb[d][:, qg * 512 + a2 * P:qg * 512 + (a2 + 1) * P],
                            xres[d][:, t * 512:(t + 1) * 512],
                            start=(d == 0), stop=(d == ND - 1))
                        if d == ND - 1:
                            sl = slice(t * 512, (t + 1) * 512)
                            rope(state["pts"][t], True, sl, q_sb[(qt, t)])
                        state["i"] += 1

                return emit_n

            # blocks: block 0 = proj(0) alone; block b>=1 = proj(b) + C(b-1).
            # pre() is emitted ONE slot ahead of post() (and the next pair's
            # pre(0) at block end) so the boundary AV never waits on exp.
            if cut == "noattn":
                for qt in range(8):
                    emit_n = proj_chunks(qt)
                    emit_n(32)
                    load_wo(qt)
                # anchor every q_sb tile so nothing is dead-code-eliminated
                for i in range(8):
                    for t in (0, 1):
                        nc.sync.dma_start(
                            out=out_d[i * P:(i + 1) * P, t * 512:(t + 1) * 512],
                            in_=q_sb[(i, t)][:])
                return

            FILLER = [5, 5, 4, 4, 4, 4, 3, 3]
            prev = None
            for qt in range(8):
                emit_n = proj_chunks(qt)
                nxt = head_pair_steps(qt)
                if prev is None:
                    emit_n(32)
                else:
                    pre, post, finish = prev
                    for j in range(8):
                        if j < 7:
                            pre(j + 1)
                        emit_n(FILLER[j])
                        post(j)
                    emit_n(32)  # drain any remainder
                    nxt[0](0)
                    finish()
                if prev is None:
                    nxt[0](0)
                prev = nxt
                load_wo(qt)

            # pair 7: E-chain filler
            pre, post, finish = prev
            e_chains = []  # sc-major: chains reading the t0 o-halves first
            for sc in range(8):
                for ot in range(4):
                    e_chains.append((ot, sc))

            def e_chain(idx, pool, upto):
                ot, sc = e_chains[idx]
                pe = pool.tile([P, 512], f32, tag="op" if pool is psOp else
                               ("pp" if pool is psP else "sp"),
                               name=pfx + f"pe{ot}_{sc}")
                for dt_ in range(upto):
                    nc.tensor.matmul(
                        pe[:], o_sb[(dt_, sc // 4)][:, (sc % 4) * P:(sc % 4 + 1) * P],
                        wo_sb[dt_][:, ot * 512:(ot + 1) * 512],
                        start=(dt_ == 0), stop=(dt_ == 7))
                return pe

            # Output staging: one [P, 2048] tile per sequence block, stored
            # as two [P, 1024] halves split across the SP and ACT HWDGE
            # rings. Small per-(ot,sc) stores serialized ~2us each on one
            # ring (HBM write-completion receipt) and dominated the tail.
            obsc = {}

            def emit_quadrant(sc, ot, write_fn):
                key = (sc, ot // 2)
                if key not in obsc:
                    obsc[key] = ob_pool.tile([P, 1024], bf16, tag="ob",
                                             name=pfx + f"obsc{sc}_{ot // 2}")
                ob = obsc[key]
                write_fn(ob[:, (ot % 2) * 512:(ot % 2) * 512 + 512])
                if ot % 2 == 1:
                    lo = (ot // 2) * 1024
                    eng = nc.sync if (sc + ot) % 2 else nc.scalar
                    eng.dma_start(
                        out=out_d[sc * P:(sc + 1) * P, lo:lo + 1024],
                        in_=ob[:])

            def e_finish(pe, idx):
                ot, sc = e_chains[idx]
                nc.tensor.matmul(
                    pe[:], o_sb[(7, sc // 4)][:, (sc % 4) * P:(sc % 4 + 1) * P],
                    wo_sb[7][:, ot * 512:(ot + 1) * 512], start=False, stop=True)
                if (ot + sc) % 2:
                    emit_quadrant(sc, ot, lambda dst: nc.scalar.copy(dst, pe[:]))
                else:
                    emit_quadrant(sc, ot,
                                  lambda dst: nc.vector.tensor_copy(dst, pe[:]))

            # filler chains 0-5 ride psP during pair 7's j-loop: accumulate
            # dt0..6, spill the partial to SBUF (releasing the psP bank for
            # the next filler chain), and add the dt7 term at the end via a
            # 1-step matmul + DVE tensor_add. pre runs TWO slots ahead here
            # (no proj filler left, so the exps must queue earlier on ACT).
            ep_pool = stack.enter_context(
                tc.tile_pool(name=pfx + "epart", bufs=6))
            parts = {}
            pre(1)
            for j in range(8):
                if j < 6:
                    pre(j + 2)
                if j < 6:
                    ot, sc = e_chains[j]
                    peh = psP.tile([P, 512], f32, tag="pp",
                                   name=pfx + f"pef{j}")
                    for dt_ in range(7):
                        nc.tensor.matmul(
                            peh[:],
                            o_sb[(dt_, sc // 4)][:, (sc % 4) * P:(sc % 4 + 1) * P],
                            wo_sb[dt_][:, ot * 512:(ot + 1) * 512],
                            start=(dt_ == 0), stop=(dt_ == 6))
                    part = ep_pool.tile([P, 512], bf16, tag="ep",
                                        name=pfx + f"ep{j}")
                    nc.vector.tensor_copy(part[:], peh[:])
                    parts[j] = part
                post(j)
            finish()
            for idx in range(6):
                ot, sc = e_chains[idx]
                peh = psP.tile([P, 512], f32, tag="pp", name=pfx + f"pet{idx}")
                nc.tensor.matmul(
                    peh[:], o_sb[(7, sc // 4)][:, (sc % 4) * P:(sc % 4 + 1) * P],
                    wo_sb[7][:, ot * 512:(ot + 1) * 512], start=True, stop=True)
                emit_quadrant(
                    sc, ot,
                    lambda dst, peh=peh, part=parts[idx]:
                        nc.vector.tensor_add(dst, peh[:], part[:]))
            if cut == "notail":
                # anchor every o_sb tile so nothing is dead-code-eliminated
                for i in range(8):
                    for t in (0, 1):
                        nc.sync.dma_start(
                            out=out_d[i * P:(i + 1) * P,
                                      1024 + t * 512:1024 + (t + 1) * 512],
                            in_=o_sb[(i, t)][:])
                return
            for idx in range(6, 31):
                pool = psS if idx % 3 != 0 else psOp
                pe = e_chain(idx, pool, 7)
                e_finish(pe, idx)
            # final chain as two parallel half-chains on the free psP banks;
            # their copies complete the sc7 staging tile, then the last
            # [P, 1024] half-store goes out
            ot31, sc31 = e_chains[31]
            hcopies = []
            for half in (0, 1):
                c0 = ot31 * 512 + half * 256
                peh = psP.tile([P, 256], f32, tag="pp",
                               name=pfx + f"peh{half}")
                for dt_ in range(8):
                    nc.tensor.matmul(
                        peh[:],
                        o_sb[(dt_, sc31 // 4)][:, (sc31 % 4) * P:(sc31 % 4 + 1) * P],
                        wo_sb[dt_][:, c0:c0 + 256],
                        start=(dt_ == 0), stop=(dt_ == 7))
                hcopies.append((c0, peh))
            ob7 = obsc[(sc31, 1)]
            base = (ot31 % 2) * 512
            nc.vector.tensor_copy(ob7[:, base:base + 256], hcopies[0][1][:])
            nc.scalar.copy(ob7[:, base + 256:base + 512], hcopies[1][1][:])
            nc.scalar.dma_start(
                out=out_d[sc31 * P:(sc31 + 1) * P, 1024:2048],
                in_=ob7[:])

    with tile.TileContext(nc) as tc:
        for rep in range(reps):
            emit(tc, f"r{rep}_" if reps > 1 else "")

    nc.compile()
    return nc


def _get_nc():
    if "nc" not in _CACHE:
        _CACHE["nc"] = build_nc()
    return _CACHE["nc"]


def kernel(**inputs):
    from concourse.bass_utils import run_bass_kernel_spmd
    nc = _get_nc()
    in_maps = host_prep(**inputs)
    res = run_bass_kernel_spmd(nc, in_maps, core_ids=list(range(8)))
    outs = [np.asarray(res.results[c]["out"], np.float32) for c in range(8)]
    full = np.stack([outs[2 * b] + outs[2 * b + 1] for b in range(B)])
    return full.astype(np.float32)


if __name__ == "__main__":
    nc = build_nc()
    print("build ok")
